# revision 39
# baseline (speedup 1.0000x reference)
# Trainium2 Bass kernel for nn_CrossAttention (B=1, I=J=1024, C_S=1024,
# C_Z=128, H=16, D=64), sharded over the query dim i across 8 NeuronCores.
#
# v8, 162.3us on HW (from v4 baseline at 173.6us; a v5 experiment that
# accumulated z into the qk PSUM via identity-stationary matmuls serialized
# the PE's weight loads — alternating stationaries with sub-LDW-length
# streams runs ~190ns/matmul — and regressed to 225us, so scores keep the
# DVE add):
#   - attention heads are software-pipelined: the o-matmuls / reciprocal /
#     o-scale of head h-1 are emitted inside head h's slot so no engine's
#     in-order stream blocks the next head's score adds (v4 ran heads at
#     ~3.2us chain latency); the o-scale rides an ACT scaled copy.
#   - z chunks all complete during the projection phase (bias DMA has
#     8-deep prefetch, 3 chunks interleaved per k-proj feature block) so
#     attention starts as soon as projections finish instead of after a
#     late z tail; wq's DMA is issued after the first bias chunks.
#   - v's second feature half is interleaved into the attention phase to
#     keep the PE busy while ACT does exps; the g projection (and both
#     sigmoids) run before the first exp (ACT table reloads cost 1.3us).
#
# kernel(**inputs) takes FULL inputs, shards on host, runs SPMD on cores 0-7,
# gathers to the full [1, 1024, 1024] output.

import numpy as np

B, I, J, CS, CZ, H, D = 1, 1024, 1024, 1024, 128, 16, 64
NCORES = 8
NI = I // NCORES  # 128 query rows per core
P = 128
KC = CS // P  # 8 contraction chunks
JC = J // P  # 8 key chunks
IC8 = 4  # i rows per bias chunk
NCHUNK = NI // IC8  # 32 bias chunks

_last_results = None


def _build_program():
    from contextlib import ExitStack

    import concourse.mybir as mybir
    import concourse.tile as tile
    from concourse import bacc
    from concourse.masks import make_identity

    f32 = mybir.dt.float32
    bf16 = mybir.dt.bfloat16
    fp8 = mybir.dt.float8e4
    AF = mybir.ActivationFunctionType
    ALU = mybir.AluOpType

    nc = bacc.Bacc("TRN2", target_bir_lowering=False, debug=False)

    # ---- dram io (host-prepared layouts, all partition-major) ----
    sT_d = nc.dram_tensor("sT", [P, KC, NI], bf16, kind="ExternalInput").ap()
    kinT_d = nc.dram_tensor("kinT", [P, KC, J], bf16, kind="ExternalInput").ap()
    biasT_d = nc.dram_tensor("biasT", [P, NI, J], fp8, kind="ExternalInput").ap()
    wqT_d = nc.dram_tensor("wqT", [P, KC, CS], bf16, kind="ExternalInput").ap()
    wkT_d = nc.dram_tensor("wkT", [P, KC, CS], bf16, kind="ExternalInput").ap()
    wvT_d = nc.dram_tensor("wvT", [P, KC, CS], bf16, kind="ExternalInput").ap()
    wgT_d = nc.dram_tensor("wgT", [P, KC, CS], bf16, kind="ExternalInput").ap()
    woT_d = nc.dram_tensor("woT", [P, KC, CS], bf16, kind="ExternalInput").ap()
    wz_d = nc.dram_tensor("w_z", [CZ, H], bf16, kind="ExternalInput").ap()
    bq_d = nc.dram_tensor("b_q", [P, KC], f32, kind="ExternalInput").ap()
    mask_d = nc.dram_tensor("mask", [P, JC], f32, kind="ExternalInput").ap()
    out_d = nc.dram_tensor("out", [NI, CS], f32, kind="ExternalOutput").ap()

    with tile.TileContext(nc) as tc, ExitStack() as ctx:
        pool = lambda name, bufs: ctx.enter_context(tc.tile_pool(name=name, bufs=bufs))
        ppool = lambda name, bufs: ctx.enter_context(
            tc.tile_pool(name=name, bufs=bufs, space="PSUM")
        )

        const = pool("const", 1)
        act_p = pool("act", 1)  # persistent small activations
        big_p = pool("big", 1)  # persistent big tensors (kinT, kT, v, z)
        bstage_p = pool("bstage", 12)  # bias^T chunks (12-deep prefetch)
        wstage_p = pool("wstage", 2)  # weight chunks
        et_p = pool("et", 4)
        st_p = pool("st", 3)
        outs_p = pool("outs", 2)

        big_ps = ppool("bigps", 2)  # [128,512] f32: projections / o-proj / go-T
        zq_ps = ppool("zqps", 4)  # [128,512] f32: z accumulation, then qk banks
        op_ps = ppool("ops", 2)  # [128,65] f32: o accumulators

        def copy_on(eng_is_vector, out, in_):
            if eng_is_vector:
                nc.vector.tensor_copy(out, in_)
            else:
                nc.scalar.copy(out, in_)

        # ---- constants / small loads (sync ring) ----
        ident = const.tile([P, P], bf16)
        make_identity(nc, ident)
        wz_s = const.tile([CZ, H], bf16)
        nc.sync.dma_start(wz_s, wz_d)

        def load_w(w_ap, tag, split=1):
            # split>1 issues the transfer as dim-1 slices so consumers of the
            # first slices can start before the whole weight has landed
            w = wstage_p.tile([P, KC, CS], bf16, tag="w", name=tag)
            step = KC // split
            for s in range(split):
                nc.scalar.dma_start(
                    w[:, s * step : (s + 1) * step, :],
                    w_ap[:, s * step : (s + 1) * step, :],
                )
            return w

        # ---- z: bias^T chunks (plain DMA) + per-(i, jc) matmuls ----
        # z_s layout: [j_part, jc, h, i] (bf16) -- i contiguous for the
        # identity-matmul accumulate in the attention inner loop
        z_s = big_p.tile([P, JC, H, NI], bf16, tag="z")

        def z_chunk(i0, ni=IC8):
            # ni i-rows per chunk (the first chunks are small so z can start
            # while the DMA engines are still warming up)
            bt = bstage_p.tile([P, IC8, J], fp8, tag="bt", name=f"bt_{i0}")
            nc.sync.dma_start(bt[:, :ni, :], biasT_d[:, i0 : i0 + ni, :])
            # up to 8 jc in one psum bank: [j=128, (8 jc, ni i, 16 h)]
            zp = zq_ps.tile([P, 512], f32, tag="zq", name=f"zp_{i0}")
            for jc in range(JC):
                for il in range(ni):
                    nc.tensor.matmul(
                        zp[:, (jc * ni + il) * H : (jc * ni + il + 1) * H],
                        bt[:, il, jc * P : (jc + 1) * P],
                        wz_s,
                        start=True,
                        stop=True,
                    )
            nc.vector.tensor_copy(
                z_s[:, :, :, i0 : i0 + ni],
                zp[:, : JC * ni * H].rearrange("p (a b c) -> p a c b", a=JC, b=ni),
            )

        # prefetch the first bias chunks hard (small ones first so z compute
        # starts during DMA warmup); wq is issued only after them so the bias
        # DMA owns the engines while the PE warms up on z. The very first
        # i-row's DMA is split in half so its first 4 matmuls start even
        # sooner.
        bt0 = bstage_p.tile([P, IC8, J], fp8, tag="bt", name="bt_0")
        nc.sync.dma_start(bt0[:, 0:1, 0:512], biasT_d[:, 0:1, 0:512])
        nc.sync.dma_start(bt0[:, 0:1, 512:J], biasT_d[:, 0:1, 512:J])
        zp0 = zq_ps.tile([P, 512], f32, tag="zq", name="zp_0")
        for jc in range(JC):
            nc.tensor.matmul(
                zp0[:, jc * H : (jc + 1) * H],
                bt0[:, 0, jc * P : (jc + 1) * P],
                wz_s,
                start=True,
                stop=True,
            )
        nc.vector.tensor_copy(
            z_s[:, :, :, 0:1],
            zp0[:, : JC * H].rearrange("p (a b c) -> p a c b", a=JC, b=1),
        )
        z_chunk(1, 1)
        z_chunk(2, 1)
        z_chunk(3, 1)
        z_chunk(4, 2)
        z_chunk(6, 2)
        wq_s = load_w(wqT_d, "wq", split=2)
        sT_s = act_p.tile([P, KC, NI], bf16, tag="sT")
        nc.sync.dma_start(sT_s, sT_d)
        kinT_s = big_p.tile([P, KC, J], bf16, tag="kinT")
        nc.sync.dma_start(kinT_s, kinT_d)
        bq_s = const.tile([P, KC], f32)
        nc.sync.dma_start(bq_s, bq_d)
        mask_s = const.tile([P, JC], f32)
        nc.sync.dma_start(mask_s, mask_d)

        # ---- q projection: qT [f, i] = Wq s^T (+bq, /sqrt(D)) ----
        qT_s = act_p.tile([P, KC, NI], bf16, tag="qT")

        def q_proj():
            for fh in range(2):
                ps = big_ps.tile([P, 512], f32, tag="big", name=f"qp_{fh}")
                for fol in range(4):
                    fo = fh * 4 + fol
                    for co in range(KC):
                        nc.tensor.matmul(
                            ps[:, fol * P : (fol + 1) * P],
                            wq_s[:, co, fo * P : (fo + 1) * P],
                            sT_s[:, co, :],
                            start=(co == 0),
                            stop=(co == KC - 1),
                        )
                for fol in range(4):
                    fo = fh * 4 + fol
                    nc.vector.tensor_scalar(
                        qT_s[:, fo, :],
                        ps[:, fol * P : (fol + 1) * P],
                        bq_s[:, fo : fo + 1],
                        1.0 / np.sqrt(D),
                        ALU.add,
                        ALU.mult,
                    )

        q_proj()
        # z fills the PE while wk's DMA completes (wk shares engines with bias)
        for k in range(7):
            z_chunk(8 + 4 * k)

        # ---- k projection: kT [f, j] = Wk k_in^T ----
        # wk arrives fo-major ([p, fo, co, 128] on the host) in 4 slices so
        # fo=0 starts after 1/4 of the weight lands; 2 z chunks per fo keep
        # the bias DMA streaming at full rate
        wk_s = load_w(wkT_d, "wk", split=4)
        kT_s = big_p.tile([P, KC, J], bf16, tag="kT")
        for fo in range(KC):
            for jh in range(2):
                ps = big_ps.tile([P, 512], f32, tag="big", name=f"kp_{fo}_{jh}")
                for co in range(KC):
                    nc.tensor.matmul(
                        ps,
                        wk_s[:, fo, co * P : (co + 1) * P],
                        kinT_s[:, co, jh * 512 : (jh + 1) * 512],
                        start=(co == 0),
                        stop=(co == KC - 1),
                    )
                copy_on(jh == 0, kT_s[:, fo, jh * 512 : (jh + 1) * 512], ps)
            z_chunk(36 + 8 * fo)
            z_chunk(40 + 8 * fo)

        # ---- g projection (early: both sigmoids before any exp) ----
        # wg arrives fh-major ([p, fh, co, 512] on the host) in 2 slices
        wg_s = load_w(wgT_d, "wg", split=2)
        g_s = act_p.tile([P, CS], bf16, tag="g")
        for fh in range(2):
            ps = big_ps.tile([P, 512], f32, tag="big", name=f"gp_{fh}")
            for co in range(KC):
                nc.tensor.matmul(
                    ps,
                    sT_s[:, co, :],
                    wg_s[:, fh * 4 + co // 2, (co % 2) * 512 : (co % 2) * 512 + 512],
                    start=(co == 0),
                    stop=(co == KC - 1),
                )
            nc.scalar.activation(g_s[:, fh * 512 : (fh + 1) * 512], ps, AF.Sigmoid)
            z_chunk(100 + 4 * fh)

        # ---- v projection: v [j, h, d|ones] = k_in Wv^T, masked ----
        # fh=0 (heads 0-7) before attention; fh=1 interleaved into it.
        # wv arrives fh-major in 2 slices like wg.
        wv_s = load_w(wvT_d, "wv", split=2)
        v_s = big_p.tile([P, JC, H, D + 1], bf16, tag="v")

        def v_block(fh, jo):
            ps = big_ps.tile([P, 512], f32, tag="big", name=f"vp_{jo}_{fh}")
            for co in range(KC):
                nc.tensor.matmul(
                    ps,
                    kinT_s[:, co, jo * P : (jo + 1) * P],
                    wv_s[:, fh * 4 + co // 2, (co % 2) * 512 : (co % 2) * 512 + 512],
                    start=(co == 0),
                    stop=(co == KC - 1),
                )
            if fh == 0:
                # pre-attention: DVE has slack here
                nc.vector.tensor_scalar_mul(
                    v_s[:, jo, 0:8, 0:D],
                    ps,
                    mask_s[:, jo : jo + 1],
                )
                nc.vector.tensor_copy(
                    v_s[:, jo, :, D : D + 1],
                    mask_s[:, jo : jo + 1, None].to_broadcast((P, H, 1)),
                )
            else:
                # in-attention: DVE is the head-cadence bottleneck (2 score
                # adds + reciprocal + alternating o-scale), so this drain
                # rides an ACT masked copy instead
                nc.scalar.activation(
                    v_s[:, jo, 8:16, 0:D],
                    ps,
                    AF.Copy,
                    scale=mask_s[:, jo : jo + 1],
                )

        for jo in range(JC):
            v_block(0, jo)
            if jo < 5:
                z_chunk(108 + 4 * jo)

        wo_s = load_w(woT_d, "wo")

        # ---- attention: j-major scores + z via identity matmul in PSUM ----
        # Per head h: qk matmuls into a [128,512] bank (4 key-chunks), one
        # z accumulate matmul (identity stationary, N=512), one exp per bank.
        # o/reciprocal/scale for head h-1 are emitted inside head h's slot.
        o_s = act_p.tile([P, CS], bf16, tag="o")
        goT = act_p.tile([P, KC, NI], bf16, tag="goT")

        op_tiles = [None] * H
        rec_tiles = [None] * H

        def scores(h):
            # both 512-wide qk banks feed ONE st tile so a single exp covers
            # the head (the per-ACT-instruction fixed cost is ~270ns)
            fo, pb = h // 2, (h % 2) * D
            st = st_p.tile([P, JC, P], f32, tag="st", name=f"st_{h}")
            for jh in range(2):
                qk = zq_ps.tile([P, 512], f32, tag="zq", name=f"qk_{h}_{jh}")
                for jcl in range(4):
                    jc = jh * 4 + jcl
                    nc.tensor.matmul(
                        qk[:, jcl * P : (jcl + 1) * P],
                        kT_s[pb : pb + D, fo, jc * P : (jc + 1) * P],
                        qT_s[pb : pb + D, fo, :],
                        start=True,
                        stop=True,
                    )
                nc.vector.tensor_tensor(
                    st[:, jh * 4 : (jh + 1) * 4, :],
                    qk.rearrange("p (a b) -> p a b", a=4),
                    z_s[:, jh * 4 : (jh + 1) * 4, h, :],
                    ALU.add,
                )
            et = et_p.tile([P, JC, P], bf16, tag="et", name=f"et_{h}")
            nc.scalar.activation(et, st, AF.Exp)
            return et

        def o_accum(h, et):
            op = op_ps.tile([P, D + 1], f32, tag="op", name=f"op_{h}")
            for jc in range(JC):
                nc.tensor.matmul(
                    op,
                    et[:, jc, :],
                    v_s[:, jc, h, :],
                    start=(jc == 0),
                    stop=(jc == JC - 1),
                )
            op_tiles[h] = op

        def o_finish(h):
            # reciprocal on DVE; the o-scale alternates between an ACT scaled
            # copy and a DVE multiply so neither engine's stream becomes the
            # attention-pipeline bottleneck
            op = op_tiles[h]
            rec = et_p.tile([P, 1], f32, tag="rec", name=f"rec_{h}")
            nc.vector.reciprocal(rec, op[:, D : D + 1])
            if h % 2 == 0:
                nc.scalar.activation(
                    o_s[:, h * D : (h + 1) * D], op[:, 0:D], AF.Copy, scale=rec
                )
            else:
                nc.vector.tensor_scalar_mul(
                    o_s[:, h * D : (h + 1) * D], op[:, 0:D], rec
                )

        def gate_half(gh):
            # gate + transpose this half while later heads proceed (tb lives
            # in the qk psum pool so big_ps can hold the out-proj partials)
            nc.vector.tensor_mul(
                g_s[:, gh * 512 : (gh + 1) * 512],
                g_s[:, gh * 512 : (gh + 1) * 512],
                o_s[:, gh * 512 : (gh + 1) * 512],
            )
            tb = zq_ps.tile([P, 512], bf16, tag="zq", name=f"tb_{gh}")
            for fo in range(gh * 4, gh * 4 + 4):
                nc.tensor.transpose(
                    tb[:, (fo % 4) * P : (fo % 4 + 1) * P],
                    g_s[:, fo * P : (fo + 1) * P],
                    ident,
                )
            nc.vector.tensor_copy(goT[:, gh * 4 : (gh + 1) * 4, :], tb)

        # out-projection accumulates in two fo-halves: fo 0-3 run during
        # attention (right after gate_half(0)), fo 4-7 in the tail
        out_ps = [None, None]

        def out_proj(part):
            for fh in range(2):
                if part == 0:
                    out_ps[fh] = big_ps.tile(
                        [P, 512], f32, tag="big", name=f"op_ps_{fh}"
                    )
                ps = out_ps[fh]
                for fo in range(part * 4, part * 4 + 4):
                    nc.tensor.matmul(
                        ps,
                        goT[:, fo, :],
                        wo_s[:, fo, fh * 512 : (fh + 1) * 512],
                        start=(fo == 0),
                        stop=(fo == KC - 1),
                    )
                if part == 1:
                    out_s = outs_p.tile([P, 512], f32, tag="outs", name=f"out_s{fh}")
                    nc.scalar.copy(out_s, ps)
                    nc.sync.dma_start(out_d[:, fh * 512 : (fh + 1) * 512], out_s)

        prev_et = None
        for h in range(H):
            et = scores(h)
            if h > 0:
                o_accum(h - 1, prev_et)
                o_finish(h - 1)
            prev_et = et
            # PE fillers while ACT runs exps: v second half, then gate halves
            if h < 6:
                v_block(1, h + 2)
            elif h == 6:
                v_block(1, 0)
                v_block(1, 1)
            if h == 9:
                gate_half(0)
            if h == 10:
                out_proj(0)
        o_accum(H - 1, prev_et)
        o_finish(H - 1)
        gate_half(1)
        out_proj(1)

    nc.compile()
    return nc


def _chunk128(a):
    # [n*128, m...] -> [128, n, m...] matching rearrange("(co p) m -> p co m")
    n = a.shape[0] // P
    return np.ascontiguousarray(a.reshape(n, P, -1).transpose(1, 0, 2))


def kernel(**inputs):
    global _last_results
    import ml_dtypes
    from concourse.bass_utils import run_bass_kernel_spmd

    bf = ml_dtypes.bfloat16
    s = np.asarray(inputs["s"], dtype=np.float32)[0]
    k_in = np.asarray(inputs["k_in"], dtype=np.float32)[0]
    mask = np.asarray(inputs["mask"], dtype=np.float32)[0]
    bias = np.asarray(inputs["bias"], dtype=np.float32)[0]
    bq = np.asarray(inputs["bq"], dtype=np.float32)
    mult = int(np.asarray(inputs.get("multiplicity", 1)))
    assert mult == 1, f"multiplicity={mult} not supported (B=1)"

    # host-side layout prep (cheap vs device HBM savings)
    sT = _chunk128(s.T.astype(bf))  # [p, co, i_full]
    kinT = _chunk128(k_in.T.astype(bf))  # [p, co, j]
    wT = {
        k: _chunk128(np.asarray(inputs[k], np.float32).T.astype(bf))
        for k in ("Wq", "Wk", "Wv", "Wg", "Wo")
    }
    # wk fo-major ([p, fo, co, 128]) so k-proj fo-blocks start on partial
    # weight arrival; wg/wv fh-major ([p, fh, co, 512]) likewise
    wT["Wk"] = np.ascontiguousarray(
        wT["Wk"].reshape(P, KC, KC, P).transpose(0, 2, 1, 3).reshape(P, KC, CS)
    )
    for k in ("Wg", "Wv"):
        wT[k] = np.ascontiguousarray(
            wT[k].reshape(P, KC, 2, 512).transpose(0, 2, 1, 3).reshape(P, KC, CS)
        )
    wz = np.ascontiguousarray(np.asarray(inputs["Wz"], np.float32).astype(bf))
    bq_r = np.ascontiguousarray(bq.reshape(KC, P).T)  # [p, fo] f32
    mask_r = np.ascontiguousarray(mask.reshape(JC, P).T)  # [p, jo] f32
    bias_q = bias.astype(ml_dtypes.float8_e4m3)  # [i_full, j, c]

    nc = _build_program()

    in_maps = []
    for c in range(NCORES):
        # bias^T per core: [c=128, i=128, j=1024]
        biasT = np.ascontiguousarray(
            bias_q[c * NI : (c + 1) * NI].transpose(2, 0, 1)
        )
        in_maps.append(
            {
                "sT": np.ascontiguousarray(sT[:, :, c * NI : (c + 1) * NI]),
                "kinT": kinT,
                "biasT": biasT,
                "wqT": wT["Wq"],
                "wkT": wT["Wk"],
                "wvT": wT["Wv"],
                "wgT": wT["Wg"],
                "woT": wT["Wo"],
                "w_z": wz,
                "b_q": bq_r,
                "mask": mask_r,
            }
        )

    try:
        res = run_bass_kernel_spmd(nc, in_maps, core_ids=list(range(NCORES)))
    except Exception:
        # transient device-unrecoverable errors have been observed on a
        # first attempt; one retry has always succeeded
        import time as _time

        _time.sleep(5.0)
        res = run_bass_kernel_spmd(nc, in_maps, core_ids=list(range(NCORES)))
    _last_results = res
    out = np.concatenate([r["out"] for r in res.results], axis=0)
    return out.reshape(B, I, CS).astype(np.float32)


if __name__ == "__main__":
    rng = np.random.default_rng(0)
    ins = {
        "s": rng.standard_normal((B, I, CS), dtype=np.float32),
        "k_in": rng.standard_normal((B, J, CS), dtype=np.float32),
        "mask": np.ones((B, J), np.float32),
        "bias": rng.standard_normal((B, I, J, CZ), dtype=np.float32),
        "Wq": rng.standard_normal((CS, CS), dtype=np.float32) * 0.02,
        "bq": rng.standard_normal((CS,), dtype=np.float32) * 0.02,
        "Wk": rng.standard_normal((CS, CS), dtype=np.float32) * 0.02,
        "Wv": rng.standard_normal((CS, CS), dtype=np.float32) * 0.02,
        "Wg": rng.standard_normal((CS, CS), dtype=np.float32) * 0.02,
        "Wo": rng.standard_normal((CS, CS), dtype=np.float32) * 0.02,
        "Wz": rng.standard_normal((CZ, H), dtype=np.float32) * 0.02,
        "multiplicity": 1,
    }
    out = kernel(**ins)
    print(out.shape, out.dtype)


# revision 47
# speedup vs baseline: 1.0658x; 1.0658x over previous
# Trainium2 Bass kernel for nn_CrossAttention (B=1, I=J=1024, C_S=1024,
# C_Z=128, H=16, D=64), sharded over the query dim i across 8 NeuronCores.
#
# v8, 162.3us on HW (from v4 baseline at 173.6us; a v5 experiment that
# accumulated z into the qk PSUM via identity-stationary matmuls serialized
# the PE's weight loads — alternating stationaries with sub-LDW-length
# streams runs ~190ns/matmul — and regressed to 225us, so scores keep the
# DVE add):
#   - attention heads are software-pipelined: the o-matmuls / reciprocal /
#     o-scale of head h-1 are emitted inside head h's slot so no engine's
#     in-order stream blocks the next head's score adds (v4 ran heads at
#     ~3.2us chain latency); the o-scale rides an ACT scaled copy.
#   - z chunks all complete during the projection phase (bias DMA has
#     8-deep prefetch, 3 chunks interleaved per k-proj feature block) so
#     attention starts as soon as projections finish instead of after a
#     late z tail; wq's DMA is issued after the first bias chunks.
#   - v's second feature half is interleaved into the attention phase to
#     keep the PE busy while ACT does exps; the g projection (and both
#     sigmoids) run before the first exp (ACT table reloads cost 1.3us).
#
# kernel(**inputs) takes FULL inputs, shards on host, runs SPMD on cores 0-7,
# gathers to the full [1, 1024, 1024] output.

import numpy as np

B, I, J, CS, CZ, H, D = 1, 1024, 1024, 1024, 128, 16, 64
NCORES = 8
NI = I // NCORES  # 128 query rows per core
P = 128
KC = CS // P  # 8 contraction chunks
JC = J // P  # 8 key chunks
IC8 = 4  # i rows per bias chunk
NCHUNK = NI // IC8  # 32 bias chunks

_last_results = None


def _build_program():
    from contextlib import ExitStack

    import concourse.mybir as mybir
    import concourse.tile as tile
    from concourse import bacc
    from concourse.masks import make_identity

    f32 = mybir.dt.float32
    bf16 = mybir.dt.bfloat16
    fp8 = mybir.dt.float8e4
    AF = mybir.ActivationFunctionType
    ALU = mybir.AluOpType

    nc = bacc.Bacc("TRN2", target_bir_lowering=False, debug=False)

    # ---- dram io (host-prepared layouts, all partition-major) ----
    sT_d = nc.dram_tensor("sT", [P, KC, NI], bf16, kind="ExternalInput").ap()
    kinT_d = nc.dram_tensor("kinT", [P, KC, J], bf16, kind="ExternalInput").ap()
    biasT_d = nc.dram_tensor("biasT", [P, NI, J], fp8, kind="ExternalInput").ap()
    wqT_d = nc.dram_tensor("wqT", [P, KC, CS], bf16, kind="ExternalInput").ap()
    wkT_d = nc.dram_tensor("wkT", [P, KC, CS], bf16, kind="ExternalInput").ap()
    wvT_d = nc.dram_tensor("wvT", [P, KC, CS], bf16, kind="ExternalInput").ap()
    wgT_d = nc.dram_tensor("wgT", [P, KC, CS], bf16, kind="ExternalInput").ap()
    woT_d = nc.dram_tensor("woT", [P, KC, CS], bf16, kind="ExternalInput").ap()
    wz_d = nc.dram_tensor("w_z", [CZ, H], bf16, kind="ExternalInput").ap()
    bq_d = nc.dram_tensor("b_q", [P, KC], f32, kind="ExternalInput").ap()
    mask_d = nc.dram_tensor("mask", [P, JC], f32, kind="ExternalInput").ap()
    out_d = nc.dram_tensor("out", [NI, CS], f32, kind="ExternalOutput").ap()

    with tile.TileContext(nc) as tc, ExitStack() as ctx:
        pool = lambda name, bufs: ctx.enter_context(tc.tile_pool(name=name, bufs=bufs))
        ppool = lambda name, bufs: ctx.enter_context(
            tc.tile_pool(name=name, bufs=bufs, space="PSUM")
        )

        const = pool("const", 1)
        act_p = pool("act", 1)  # persistent small activations
        big_p = pool("big", 1)  # persistent big tensors (kinT, kT, v, z)
        bstage_p = pool("bstage", 12)  # bias^T chunks (12-deep prefetch)
        wstage_p = pool("wstage", 2)  # weight chunks
        et_p = pool("et", 4)
        st_p = pool("st", 3)
        outs_p = pool("outs", 2)

        big_ps = ppool("bigps", 2)  # [128,512] f32: projections / o-proj / go-T
        zq_ps = ppool("zqps", 4)  # [128,512] f32: z accumulation, then qk banks
        op_ps = ppool("ops", 2)  # [128,65] f32: o accumulators

        def copy_on(eng_is_vector, out, in_):
            if eng_is_vector:
                nc.vector.tensor_copy(out, in_)
            else:
                nc.scalar.copy(out, in_)

        # ---- constants / small loads (sync ring) ----
        ident = const.tile([P, P], bf16)
        make_identity(nc, ident)
        wz_s = const.tile([CZ, H], bf16)
        nc.sync.dma_start(wz_s, wz_d)

        def load_w(w_ap, tag, split=1):
            # split>1 issues the transfer as dim-1 slices so consumers of the
            # first slices can start before the whole weight has landed
            w = wstage_p.tile([P, KC, CS], bf16, tag="w", name=tag)
            step = KC // split
            for s in range(split):
                nc.scalar.dma_start(
                    w[:, s * step : (s + 1) * step, :],
                    w_ap[:, s * step : (s + 1) * step, :],
                )
            return w

        # ---- z: bias^T chunks (plain DMA) + per-(i, jc) matmuls ----
        # z_s layout: [j_part, jc, h, i] (bf16) -- i contiguous for the
        # identity-matmul accumulate in the attention inner loop
        z_s = big_p.tile([P, JC, H, NI], bf16, tag="z")

        def z_chunk(i0, ni=IC8):
            # ni i-rows per chunk (the first chunks are small so z can start
            # while the DMA engines are still warming up)
            bt = bstage_p.tile([P, IC8, J], fp8, tag="bt", name=f"bt_{i0}")
            nc.sync.dma_start(bt[:, :ni, :], biasT_d[:, i0 : i0 + ni, :])
            # up to 8 jc in one psum bank: [j=128, (8 jc, ni i, 16 h)]
            zp = zq_ps.tile([P, 512], f32, tag="zq", name=f"zp_{i0}")
            for jc in range(JC):
                for il in range(ni):
                    nc.tensor.matmul(
                        zp[:, (jc * ni + il) * H : (jc * ni + il + 1) * H],
                        bt[:, il, jc * P : (jc + 1) * P],
                        wz_s,
                        start=True,
                        stop=True,
                    )
            nc.vector.tensor_copy(
                z_s[:, :, :, i0 : i0 + ni],
                zp[:, : JC * ni * H].rearrange("p (a b c) -> p a c b", a=JC, b=ni),
            )

        # prefetch the first bias chunks hard (small ones first so z compute
        # starts during DMA warmup); wq is issued only after them so the bias
        # DMA owns the engines while the PE warms up on z
        z_chunk(0, 1)
        z_chunk(1, 1)
        z_chunk(2, 1)
        z_chunk(3, 1)
        z_chunk(4, 2)
        z_chunk(6, 2)
        wq_s = load_w(wqT_d, "wq", split=2)
        sT_s = act_p.tile([P, KC, NI], bf16, tag="sT")
        nc.sync.dma_start(sT_s, sT_d)
        # kinT arrives jh-major ([p, jh, co, 512] on the host) in 2 slices so
        # the k projection's first j-half starts after 1.05MB instead of 2.1
        kinT_s = big_p.tile([P, KC, J], bf16, tag="kinT")
        nc.sync.dma_start(kinT_s[:, 0:4, :], kinT_d[:, 0:4, :])
        nc.sync.dma_start(kinT_s[:, 4:8, :], kinT_d[:, 4:8, :])
        bq_s = const.tile([P, KC], f32)
        nc.sync.dma_start(bq_s, bq_d)
        mask_s = const.tile([P, JC], f32)
        nc.sync.dma_start(mask_s, mask_d)

        # ---- q projection: qT [f, i] = Wq s^T (+bq, /sqrt(D)) ----
        qT_s = act_p.tile([P, KC, NI], bf16, tag="qT")

        def q_proj():
            for fh in range(2):
                ps = big_ps.tile([P, 512], f32, tag="big", name=f"qp_{fh}")
                for fol in range(4):
                    fo = fh * 4 + fol
                    for co in range(KC):
                        nc.tensor.matmul(
                            ps[:, fol * P : (fol + 1) * P],
                            wq_s[:, co, fo * P : (fo + 1) * P],
                            sT_s[:, co, :],
                            start=(co == 0),
                            stop=(co == KC - 1),
                        )
                for fol in range(4):
                    fo = fh * 4 + fol
                    nc.vector.tensor_scalar(
                        qT_s[:, fo, :],
                        ps[:, fol * P : (fol + 1) * P],
                        bq_s[:, fo : fo + 1],
                        1.0 / np.sqrt(D),
                        ALU.add,
                        ALU.mult,
                    )

        q_proj()
        # z fills the PE while wk's DMA completes (wk shares engines with bias)
        for k in range(7):
            z_chunk(8 + 4 * k)

        # ---- k projection: kT [f, j] = Wk k_in^T ----
        # wk arrives fo-major ([p, fo, co, 128] on the host) in 4 slices so
        # fo=0 starts after 1/4 of the weight lands; 2 z chunks per fo keep
        # the bias DMA streaming at full rate
        wk_s = load_w(wkT_d, "wk", split=4)
        kT_s = big_p.tile([P, KC, J], bf16, tag="kT")
        for fo in range(KC):
            for jh in range(2):
                ps = big_ps.tile([P, 512], f32, tag="big", name=f"kp_{fo}_{jh}")
                for co in range(KC):
                    nc.tensor.matmul(
                        ps,
                        wk_s[:, fo, co * P : (co + 1) * P],
                        kinT_s[
                            :,
                            jh * 4 + co // 2,
                            (co % 2) * 512 : (co % 2) * 512 + 512,
                        ],
                        start=(co == 0),
                        stop=(co == KC - 1),
                    )
                copy_on(jh == 0, kT_s[:, fo, jh * 512 : (jh + 1) * 512], ps)
            z_chunk(36 + 8 * fo)
            z_chunk(40 + 8 * fo)

        # ---- g projection (early: both sigmoids before any exp) ----
        # wg arrives fh-major ([p, fh, co, 512] on the host) in 2 slices
        wg_s = load_w(wgT_d, "wg", split=2)
        g_s = act_p.tile([P, CS], bf16, tag="g")
        for fh in range(2):
            ps = big_ps.tile([P, 512], f32, tag="big", name=f"gp_{fh}")
            for co in range(KC):
                nc.tensor.matmul(
                    ps,
                    sT_s[:, co, :],
                    wg_s[:, fh * 4 + co // 2, (co % 2) * 512 : (co % 2) * 512 + 512],
                    start=(co == 0),
                    stop=(co == KC - 1),
                )
            nc.scalar.activation(g_s[:, fh * 512 : (fh + 1) * 512], ps, AF.Sigmoid)
            z_chunk(100 + 4 * fh)

        # ---- v projection: v [j, h, d|ones] = k_in Wv^T, masked ----
        # fh=0 (heads 0-7) before attention; fh=1 interleaved into it.
        # wv arrives fh-major in 2 slices like wg.
        wv_s = load_w(wvT_d, "wv", split=2)
        v_s = big_p.tile([P, JC, H, D + 1], bf16, tag="v")

        def v_block(fh, jo):
            ps = big_ps.tile([P, 512], f32, tag="big", name=f"vp_{jo}_{fh}")
            for co in range(KC):
                nc.tensor.matmul(
                    ps,
                    kinT_s[
                        :,
                        (jo // 4) * 4 + co // 2,
                        (co % 2) * 512
                        + (jo % 4) * P : (co % 2) * 512
                        + (jo % 4) * P
                        + P,
                    ],
                    wv_s[:, fh * 4 + co // 2, (co % 2) * 512 : (co % 2) * 512 + 512],
                    start=(co == 0),
                    stop=(co == KC - 1),
                )
            nc.vector.tensor_scalar_mul(
                v_s[:, jo, fh * 8 : (fh + 1) * 8, 0:D],
                ps,
                mask_s[:, jo : jo + 1],
            )
            if fh == 0:
                nc.vector.tensor_copy(
                    v_s[:, jo, :, D : D + 1],
                    mask_s[:, jo : jo + 1, None].to_broadcast((P, H, 1)),
                )

        for jo in range(JC):
            v_block(0, jo)
            if jo < 5:
                z_chunk(108 + 4 * jo)

        wo_s = load_w(woT_d, "wo")

        # ---- attention: j-major scores + z via identity matmul in PSUM ----
        # Per head h: qk matmuls into a [128,512] bank (4 key-chunks), one
        # z accumulate matmul (identity stationary, N=512), one exp per bank.
        # o/reciprocal/scale for head h-1 are emitted inside head h's slot.
        o_s = act_p.tile([P, CS], bf16, tag="o")
        goT = act_p.tile([P, KC, NI], bf16, tag="goT")

        op_tiles = [None] * H
        rec_tiles = [None] * H

        def scores(h):
            # both 512-wide qk banks feed ONE st tile so a single exp covers
            # the head (the per-ACT-instruction fixed cost is ~270ns)
            fo, pb = h // 2, (h % 2) * D
            st = st_p.tile([P, JC, P], f32, tag="st", name=f"st_{h}")
            for jh in range(2):
                qk = zq_ps.tile([P, 512], f32, tag="zq", name=f"qk_{h}_{jh}")
                for jcl in range(4):
                    jc = jh * 4 + jcl
                    nc.tensor.matmul(
                        qk[:, jcl * P : (jcl + 1) * P],
                        kT_s[pb : pb + D, fo, jc * P : (jc + 1) * P],
                        qT_s[pb : pb + D, fo, :],
                        start=True,
                        stop=True,
                    )
                nc.vector.tensor_tensor(
                    st[:, jh * 4 : (jh + 1) * 4, :],
                    qk.rearrange("p (a b) -> p a b", a=4),
                    z_s[:, jh * 4 : (jh + 1) * 4, h, :],
                    ALU.add,
                )
            et = et_p.tile([P, JC, P], bf16, tag="et", name=f"et_{h}")
            nc.scalar.activation(et, st, AF.Exp)
            return et

        def o_accum(h, et):
            op = op_ps.tile([P, D + 1], f32, tag="op", name=f"op_{h}")
            for jc in range(JC):
                nc.tensor.matmul(
                    op,
                    et[:, jc, :],
                    v_s[:, jc, h, :],
                    start=(jc == 0),
                    stop=(jc == JC - 1),
                )
            op_tiles[h] = op

        def o_finish(h):
            # reciprocal on DVE; the o-scale alternates between an ACT scaled
            # copy and a DVE multiply so neither engine's stream becomes the
            # attention-pipeline bottleneck
            op = op_tiles[h]
            rec = et_p.tile([P, 1], f32, tag="rec", name=f"rec_{h}")
            nc.vector.reciprocal(rec, op[:, D : D + 1])
            if h % 2 == 0:
                nc.scalar.activation(
                    o_s[:, h * D : (h + 1) * D], op[:, 0:D], AF.Copy, scale=rec
                )
            else:
                nc.vector.tensor_scalar_mul(
                    o_s[:, h * D : (h + 1) * D], op[:, 0:D], rec
                )

        def gate_half(gh):
            # gate + transpose this half while later heads proceed (tb lives
            # in the qk psum pool so big_ps can hold the out-proj partials)
            nc.vector.tensor_mul(
                g_s[:, gh * 512 : (gh + 1) * 512],
                g_s[:, gh * 512 : (gh + 1) * 512],
                o_s[:, gh * 512 : (gh + 1) * 512],
            )
            tb = zq_ps.tile([P, 512], bf16, tag="zq", name=f"tb_{gh}")
            for fo in range(gh * 4, gh * 4 + 4):
                nc.tensor.transpose(
                    tb[:, (fo % 4) * P : (fo % 4 + 1) * P],
                    g_s[:, fo * P : (fo + 1) * P],
                    ident,
                )
            nc.vector.tensor_copy(goT[:, gh * 4 : (gh + 1) * 4, :], tb)

        # out-projection accumulates in two fo-halves: fo 0-3 run during
        # attention (right after gate_half(0)), fo 4-7 in the tail
        out_ps = [None, None]

        def out_proj(part):
            for fh in range(2):
                if part == 0:
                    out_ps[fh] = big_ps.tile(
                        [P, 512], f32, tag="big", name=f"op_ps_{fh}"
                    )
                ps = out_ps[fh]
                for fo in range(part * 4, part * 4 + 4):
                    nc.tensor.matmul(
                        ps,
                        goT[:, fo, :],
                        wo_s[:, fo, fh * 512 : (fh + 1) * 512],
                        start=(fo == 0),
                        stop=(fo == KC - 1),
                    )
                if part == 1:
                    out_s = outs_p.tile([P, 512], f32, tag="outs", name=f"out_s{fh}")
                    nc.scalar.copy(out_s, ps)
                    nc.sync.dma_start(out_d[:, fh * 512 : (fh + 1) * 512], out_s)

        prev_et = None
        for h in range(H):
            et = scores(h)
            if h > 0:
                o_accum(h - 1, prev_et)
                o_finish(h - 1)
            prev_et = et
            # PE fillers while ACT runs exps: v second half, then gate halves
            if h < 6:
                v_block(1, h + 2)
            elif h == 6:
                v_block(1, 0)
                v_block(1, 1)
            if h == 9:
                gate_half(0)
            if h == 10:
                out_proj(0)
        o_accum(H - 1, prev_et)
        o_finish(H - 1)
        gate_half(1)
        out_proj(1)

    nc.compile()
    return nc


def _chunk128(a):
    # [n*128, m...] -> [128, n, m...] matching rearrange("(co p) m -> p co m")
    n = a.shape[0] // P
    return np.ascontiguousarray(a.reshape(n, P, -1).transpose(1, 0, 2))


def kernel(**inputs):
    global _last_results
    import ml_dtypes
    from concourse.bass_utils import run_bass_kernel_spmd

    bf = ml_dtypes.bfloat16
    s = np.asarray(inputs["s"], dtype=np.float32)[0]
    k_in = np.asarray(inputs["k_in"], dtype=np.float32)[0]
    mask = np.asarray(inputs["mask"], dtype=np.float32)[0]
    bias = np.asarray(inputs["bias"], dtype=np.float32)[0]
    bq = np.asarray(inputs["bq"], dtype=np.float32)
    mult = int(np.asarray(inputs.get("multiplicity", 1)))
    assert mult == 1, f"multiplicity={mult} not supported (B=1)"

    # host-side layout prep (cheap vs device HBM savings)
    sT = _chunk128(s.T.astype(bf))  # [p, co, i_full]
    kinT = _chunk128(k_in.T.astype(bf))  # [p, co, j]
    wT = {
        k: _chunk128(np.asarray(inputs[k], np.float32).T.astype(bf))
        for k in ("Wq", "Wk", "Wv", "Wg", "Wo")
    }
    # wk fo-major ([p, fo, co, 128]) so k-proj fo-blocks start on partial
    # weight arrival; wg/wv fh-major ([p, fh, co, 512]) likewise; kinT
    # jh-major for the same reason
    wT["Wk"] = np.ascontiguousarray(
        wT["Wk"].reshape(P, KC, KC, P).transpose(0, 2, 1, 3).reshape(P, KC, CS)
    )
    kinT = np.ascontiguousarray(
        kinT.reshape(P, KC, 2, 512).transpose(0, 2, 1, 3).reshape(P, KC, J)
    )
    for k in ("Wg", "Wv"):
        wT[k] = np.ascontiguousarray(
            wT[k].reshape(P, KC, 2, 512).transpose(0, 2, 1, 3).reshape(P, KC, CS)
        )
    wz = np.ascontiguousarray(np.asarray(inputs["Wz"], np.float32).astype(bf))
    bq_r = np.ascontiguousarray(bq.reshape(KC, P).T)  # [p, fo] f32
    mask_r = np.ascontiguousarray(mask.reshape(JC, P).T)  # [p, jo] f32
    bias_q = bias.astype(ml_dtypes.float8_e4m3)  # [i_full, j, c]

    nc = _build_program()

    in_maps = []
    for c in range(NCORES):
        # bias^T per core: [c=128, i=128, j=1024]
        biasT = np.ascontiguousarray(
            bias_q[c * NI : (c + 1) * NI].transpose(2, 0, 1)
        )
        in_maps.append(
            {
                "sT": np.ascontiguousarray(sT[:, :, c * NI : (c + 1) * NI]),
                "kinT": kinT,
                "biasT": biasT,
                "wqT": wT["Wq"],
                "wkT": wT["Wk"],
                "wvT": wT["Wv"],
                "wgT": wT["Wg"],
                "woT": wT["Wo"],
                "w_z": wz,
                "b_q": bq_r,
                "mask": mask_r,
            }
        )

    try:
        res = run_bass_kernel_spmd(nc, in_maps, core_ids=list(range(NCORES)))
    except Exception:
        # transient device-unrecoverable errors have been observed on a
        # first attempt; one retry has always succeeded
        import time as _time

        _time.sleep(5.0)
        res = run_bass_kernel_spmd(nc, in_maps, core_ids=list(range(NCORES)))
    _last_results = res
    out = np.concatenate([r["out"] for r in res.results], axis=0)
    return out.reshape(B, I, CS).astype(np.float32)


if __name__ == "__main__":
    rng = np.random.default_rng(0)
    ins = {
        "s": rng.standard_normal((B, I, CS), dtype=np.float32),
        "k_in": rng.standard_normal((B, J, CS), dtype=np.float32),
        "mask": np.ones((B, J), np.float32),
        "bias": rng.standard_normal((B, I, J, CZ), dtype=np.float32),
        "Wq": rng.standard_normal((CS, CS), dtype=np.float32) * 0.02,
        "bq": rng.standard_normal((CS,), dtype=np.float32) * 0.02,
        "Wk": rng.standard_normal((CS, CS), dtype=np.float32) * 0.02,
        "Wv": rng.standard_normal((CS, CS), dtype=np.float32) * 0.02,
        "Wg": rng.standard_normal((CS, CS), dtype=np.float32) * 0.02,
        "Wo": rng.standard_normal((CS, CS), dtype=np.float32) * 0.02,
        "Wz": rng.standard_normal((CZ, H), dtype=np.float32) * 0.02,
        "multiplicity": 1,
    }
    out = kernel(**ins)
    print(out.shape, out.dtype)
